# revision 6
# baseline (speedup 1.0000x reference)
"""Trainium2 Bass kernel for nn_CRF_SelfAttention_49065706390003.

Math: the reference's MultiheadAttention runs with sequence length 1, so the
softmax is over a singleton axis (all ones) and ctx == v; the per-scale
multiply-by-counts / divide-by-counts cancels, so the whole module collapses
to

    out[p, f, :] = emb[f, p, :] @ G + b_eff
    G            = 0.75 * (Wmp @ Wo @ Wv).T          [2048, 64]
    b_eff        = 0.75 * Wmp @ (Wo @ bv + bo) + bmp [64]

Wq/Wk/bq/bk are mathematically dead (softmax over a length-1 axis is 1).

Sharding (per the data-parallel hint): the n_partitions axis (1024) is split
across the 8 cores (128 each -> 2304 tokens/core); the small (derived) weight
matrix G and bias are replicated. All tensor-data compute (the [18432, 2048]
x [2048, 64] token matmul over emb, >99.8% of the collapsed model's FLOPs)
runs on the NeuronCores; the constant weight fold G (weights only) is
precomputed on the host while preparing the replicated inputs.

The kernel is HBM-bandwidth-bound (358 GB/s/core): the only irreducible
traffic is reading each core's emb shard once. Activations and G are fed in
fp16 (PE-native; fp32 PSUM accumulate), halving the stream vs fp32 for a
measured end-to-end relative error of ~2.5e-4 (fp32 reference compare; bf16
would be ~2e-3, fp8 e4m3 ~4.6e-2). x-chunk DMAs are split across both
hardware DGE queues (qSP / qAct).
"""

import os
import sys

for _p in ("/opt/trn_rl_repo",):
    if _p not in sys.path and os.path.isdir(_p):
        sys.path.insert(0, _p)

from contextlib import ExitStack

import numpy as np

import bass_rust

import concourse.tile as tile
from concourse import bacc, mybir
from concourse.bass import ds, ts
from concourse.bass_utils import run_bass_kernel_spmd

F = 18        # n_frames
PTOT = 1024   # n_partitions
E = 2048      # n_hidden
C = 64        # n_cluster
NCORES = 8
PSH = PTOT // NCORES          # 128 partitions per core
NTOK = F * PSH                # 2304 tokens per core
KC = E // 128                 # 16 contraction chunks
NT = (NTOK + 511) // 512      # 5 token tiles (4x512 + 256)
F32 = mybir.dt.float32
F16 = mybir.dt.float16

DUAL_QUEUE = True             # split x loads across qSP + qAct HW DGE queues


def _grouped_src(xT, k0: int, g: int):
    """DRAM AP for chunks [k0, k0+g): [128, g, NTOK] view of xT so one DMA
    loads g contraction chunks into adjacent SBUF column blocks."""
    v = xT[ts(k0, 128), :].copy()
    v.ap = bass_rust.VecI64Pair([[NTOK, 128], [128 * NTOK, g], [1, NTOK]])
    return v


def _build(dual_queue: bool = DUAL_QUEUE):
    nc = bacc.Bacc(
        "TRN2", target_bir_lowering=False, debug=False, num_devices=NCORES,
        enable_partition_id=False,
    )
    xT = nc.dram_tensor("xT", [E, NTOK], F16, kind="ExternalInput").ap()
    # G packed: (p, k*C + c) = G[k*128 + p, c]
    gT = nc.dram_tensor("gT", [128, KC * C], F16, kind="ExternalInput").ap()
    beff_in = nc.dram_tensor("beff", [C, 1], F32, kind="ExternalInput").ap()
    outT = nc.dram_tensor("outT", [C, NTOK], F16, kind="ExternalOutput").ap()

    def q(i):
        # alternate between the two hardware DGE queues (SP / Activation)
        if dual_queue and (i % 2 == 1):
            return nc.scalar
        return nc.sync

    with tile.TileContext(nc) as tc:
        with ExitStack() as ctx:
            consts = ctx.enter_context(tc.tile_pool(name="consts", bufs=1))
            pacc = ctx.enter_context(
                tc.tile_pool(name="pacc", bufs=NT, space="PSUM")
            )

            # weights first on each queue so the PE can start as soon as the
            # first x chunk lands
            Gt_sb = consts.tile([128, KC * C], F16)
            nc.sync.dma_start(Gt_sb, gT)
            b_eff = consts.tile([C, 1], F32)
            (nc.scalar if dual_queue else nc.sync).dma_start(b_eff, beff_in)
            out_sb = consts.tile([C, NTOK], F16)

            # One flat x buffer [128, KC*NTOK] (72 KiB/partition), chunk k at
            # columns [k*NTOK, (k+1)*NTOK). Chunks 0..13 load as 7 paired
            # DMAs alternating queues (order preserved per queue so the PE's
            # in-order accumulation never waits on an out-of-order queue);
            # chunk 14 single; chunk 15 split per token tile so each tile's
            # final matmul + bias-add + store pipelines with the DMA tail.
            x_sb = consts.tile([128, KC * NTOK], F16)
            xs = [x_sb[:, ds(k * NTOK, NTOK)] for k in range(KC)]
            for i, k0 in enumerate(range(0, 14, 2)):
                q(i).dma_start(
                    x_sb[:, ds(k0 * NTOK, 2 * NTOK)], _grouped_src(xT, k0, 2)
                )
            q(1).dma_start(xs[14], xT[ts(14, 128), :])
            for j in range(NT):
                jw = min(512, NTOK - j * 512)
                q(j).dma_start(
                    xs[15][:, ds(j * 512, jw)],
                    xT[ts(15, 128), ds(j * 512, jw)],
                )

            # Column-group packing: even token tiles run on PE cols 0-63
            # (psum partitions 0:64), odd tiles on cols 64-127 — two
            # concurrent matmul streams.
            def half(bank, n, w=512):
                return bank[0:64, :w] if n % 2 == 0 else bank[64:128, :w]

            def tpos(n):
                return (0, 0) if n % 2 == 0 else (0, 64)

            po = [
                pacc.tile([128, 512], F32, tag="acc", name=f"po{j}")
                for j in range(NT)
            ]
            for k in range(KC):
                lh = Gt_sb[:, ts(k, C)]
                for j in range(NT):
                    jw = min(512, NTOK - j * 512)
                    nc.tensor.matmul(
                        half(po[j], j, jw), lh, xs[k][:, ds(j * 512, jw)],
                        start=(k == 0), stop=(k == KC - 1),
                        tile_position=tpos(j),
                    )
            # bias-add + fp16 downcast, spread over vector/scalar so the 5
            # tail ops don't serialize on one engine (gpsimd can't read
            # PSUM); stores alternate queues
            for j in range(NT):
                jw = min(512, NTOK - j * 512)
                if j % 2 == 0:
                    nc.vector.tensor_scalar_add(
                        out_sb[:, ds(j * 512, jw)], half(po[j], j, jw), b_eff
                    )
                else:
                    nc.scalar.activation(
                        out_sb[:, ds(j * 512, jw)], half(po[j], j, jw),
                        mybir.ActivationFunctionType.Identity, bias=b_eff,
                    )
                q(j).dma_start(
                    outT[:, ds(j * 512, jw)], out_sb[:, ds(j * 512, jw)]
                )

    nc.compile()
    return nc


_NC_CACHE: dict = {}


def _get_nc(dual_queue: bool = DUAL_QUEUE):
    if dual_queue not in _NC_CACHE:
        _NC_CACHE[dual_queue] = _build(dual_queue)
    return _NC_CACHE[dual_queue]


def _pack_kpc(a: np.ndarray) -> np.ndarray:
    """[KC*128, C] -> [128, KC*C] with (p, k*C+c) = a[k*128+p, c]."""
    return np.ascontiguousarray(
        a.reshape(KC, 128, C).transpose(1, 0, 2).reshape(128, KC * C)
    )


def make_in_maps(inputs: dict):
    emb = np.asarray(inputs["emb"], np.float32)
    Wv = np.asarray(inputs["Wv"], np.float32)
    Wo = np.asarray(inputs["Wo"], np.float32)
    Wmp = np.asarray(inputs["Wmp"], np.float32)
    bv = np.asarray(inputs["bv"], np.float32)
    bo = np.asarray(inputs["bo"], np.float32)
    bmp = np.asarray(inputs["bmp"], np.float32)

    T = Wmp @ Wo
    G = 0.75 * (T @ Wv).T
    beff = 0.75 * (Wmp @ (Wo @ bv + bo)) + bmp
    shared = {
        "gT": _pack_kpc(G.astype(np.float32)).astype(np.float16),
        "beff": np.ascontiguousarray(beff.astype(np.float32)[:, None]),
    }

    emb16 = emb.astype(np.float16)
    in_maps = []
    for c in range(NCORES):
        sl = emb16[:, c * PSH:(c + 1) * PSH, :].reshape(NTOK, E)
        in_maps.append({"xT": np.ascontiguousarray(sl.T), **shared})
    return in_maps


def assemble(results) -> np.ndarray:
    parts = []
    for c in range(NCORES):
        o = np.asarray(results[c]["outT"]).astype(np.float32)  # [C, NTOK]
        parts.append(o.T.reshape(F, PSH, C).transpose(1, 0, 2))
    return np.ascontiguousarray(np.concatenate(parts, axis=0))


def run(inputs: dict, dual_queue: bool = DUAL_QUEUE, **kw):
    nc = _get_nc(dual_queue)
    in_maps = make_in_maps(inputs)
    res = run_bass_kernel_spmd(nc, in_maps, list(range(NCORES)), **kw)
    return assemble(res.results), res


def kernel(**inputs) -> np.ndarray:
    out, _ = run(inputs)
    return out
